# revision 43
# baseline (speedup 1.0000x reference)
"""Hausdorff distance kernel for Trainium2 (8 NeuronCores, Bass/Tile).

Pipeline:
  host   : binary masks -> edge point sets (raster order, truncated to 32768)
           capped separable EDT -> exact per-source 1-NN upper bounds
           morton-sorted source chunks (<=128 pts) + AABB candidate pruning
           (sub-chunk granularity 4) -> chunk parts of <=2048 candidate cols
           LPT packing onto 8 cores with rank-aligned slot widths (SPMD)
  device : per slot: d^2 = phi(src) . psi(cand) via fp8e4m3 DoubleRow matmul
           (K=2x11 exact integer lift, fp16-safe pair sums)
           absorbed by one of two engine paths chosen for load balance:
             TR  - VectorE tensor_reduce (3D batched over equal-width runs)
                   -> exact per-source min d^2
             SM  - ScalarE Exp activation with per-source bias = S*ub2 and
                   sum accumulator -> stabilized softmin (err <= ln(ties)/S)
  host   : combine chunk parts (min / log-sum-exp), max per direction,
           HD = sqrt(max(h_ab, h_ba)) per batch item

d^2 is exact: coordinates are small integers, every product/sum stays below
2^24, so fp32 PSUM accumulation is exact; the SM path's softmin understates
each per-source min by at most ln(#near-ties)/S (~0.1% on HD).
"""

import os
import numpy as np

GRID = 128          # D == H == W of the voxel grid
K_MAX = 32768       # reference truncates edge sets to this many points
CH = 128            # source points per chunk (= PSUM partitions)
N_CORES = 8
EDT_CAP = 24        # per-axis cap of the host EDT used for pruning bounds
DIAG2_MAX = 400     # cut chunks when cumulative AABB diagonal^2 exceeds this
SUB = 4             # sub-chunk granularity for candidate bounds
WQ = 32             # slot width quantum (candidate columns)
PART_MAX = 1024     # max candidate columns per chunk part (one PSUM tile)
TILE_COLS = 1024    # PSUM tile columns (2 banks); 4 tiles fill PSUM
SM_SCALE = 12.0     # softmin sharpness
SM_MIN_W = 128      # softmin eligibility threshold (amortize 330ns overhead)
SENT = 999.0        # far-sentinel coordinate for padding columns

_prog_cache = {}


# ----------------------------------------------------------------- host side

def _edge_points(mask):
    """mask [D,H,W] bool -> edge points [N,3] float32, raster order, <=K_MAX."""
    D, H, W = mask.shape
    p = np.pad(mask, 1)
    neigh = np.zeros_like(mask)
    for dz in range(3):
        for dy in range(3):
            for dx in range(3):
                neigh |= p[dz:dz + D, dy:dy + H, dx:dx + W]
    edge = neigh & ~mask
    pts = np.argwhere(edge)
    return pts[:K_MAX].astype(np.float32)


def _capped_edt_sq(tgt_pts, qry_pts, cap=EDT_CAP):
    """Exact min squared distance from each query point to the target set,
    by capped separable brute-force EDT on a cropped grid. +inf where the
    nearest target is farther than `cap` on some axis."""
    allpts = np.concatenate([tgt_pts, qry_pts], 0).astype(np.int64)
    lo = allpts.min(0)
    hi = allpts.max(0) + 1
    shape = tuple((hi - lo).tolist())
    INF = np.float32(3e18)
    g = np.full(shape, INF, np.float32)
    ti = tgt_pts.astype(np.int64) - lo
    g[ti[:, 0], ti[:, 1], ti[:, 2]] = 0.0
    for ax in range(3):
        res = np.full_like(g, INF)
        n = g.shape[ax]
        for s in range(-cap, cap + 1):
            if abs(s) >= n:
                continue
            src = [slice(None)] * 3
            dst = [slice(None)] * 3
            if s >= 0:
                src[ax] = slice(0, n - s)
                dst[ax] = slice(s, None)
            else:
                src[ax] = slice(-s, None)
                dst[ax] = slice(0, n + s)
            np.minimum(res[tuple(dst)], g[tuple(src)] + np.float32(s * s),
                       out=res[tuple(dst)])
        g = res
    qi = qry_pts.astype(np.int64) - lo
    out = g[qi[:, 0], qi[:, 1], qi[:, 2]].astype(np.float64)
    out[out > 1e18] = np.inf
    return out


def _morton(pts):
    x = pts.astype(np.int64)
    code = np.zeros(len(pts), np.int64)
    for b in range(7):
        for d in range(3):
            code |= ((x[:, d] >> b) & 1) << (3 * b + d)
    return code


def _build_chunks(S, T, ub2, d_id):
    """Split morton-sorted S into compact chunks; per chunk collect candidate
    targets (AABB lower bound vs per-source exact upper bound at SUB
    granularity). Returns chunk-part dicts."""
    order = np.argsort(_morton(S), kind="stable")
    S = S[order]
    ub2 = ub2[order]
    parts = []
    chunk_id = 0
    i = 0
    N = len(S)
    while i < N:
        seg = S[i:min(i + CH, N)]
        lo = np.minimum.accumulate(seg, 0)
        hi = np.maximum.accumulate(seg, 0)
        diag2 = ((hi - lo) ** 2).sum(1)
        k = int(np.searchsorted(diag2, DIAG2_MAX, side="right"))
        k = max(min(k, len(seg)), min(32, len(seg)))
        s = S[i:i + k]
        u = ub2[i:i + k]
        mask = np.zeros(len(T), bool)
        for s0 in range(0, len(s), SUB):
            ss = s[s0:s0 + SUB]
            ub = u[s0:s0 + SUB].max()
            if not np.isfinite(ub):
                mask[:] = True
                break
            alo = ss.min(0)
            ahi = ss.max(0)
            lb2 = (np.maximum(np.maximum(alo - T, T - ahi), 0.0) ** 2).sum(1)
            mask |= lb2 <= ub
        cand = T[mask]
        nreal = len(s)
        if nreal < CH:
            s = np.concatenate([s, np.repeat(s[:1], CH - nreal, 0)], 0)
            u = np.concatenate([u, np.repeat(u[:1], CH - nreal, 0)], 0)
        finite_ub = bool(np.isfinite(u).all())
        for c0 in range(0, len(cand), PART_MAX):
            parts.append({
                "dir": d_id, "chunk": chunk_id, "src": s, "ub2": u,
                "nreal": nreal, "cand": cand[c0:c0 + PART_MAX],
                "sm_ok": finite_ub,
            })
        chunk_id += 1
        i += k
    return parts


K_LIFT = 11  # rows per DoubleRow half: d^2 as a K=22 inner product of
             # fp8-exact factors. The device fp8e4 is IEEE e4m3 (max normal
             # 240), and the DoubleRow pair-sum is rounded to fp16, so rows
             # are paired such that every pair-sum is a multiple of its fp16
             # ulp and <= 65504: exactness is preserved end to end.
             # Pairs (A_i, B_i): (S3a,S3b) (T3a,T3b) (R1x,R1y) (R1z,S2)
             # (T2,R2x) (R2y,R3x) (R2z,R3y) (R3z,S1) (R4x,R4y) (R4z,S0)
             # (T0,T1), where R1..R4 are the per-coordinate cross terms of
             # -2<s,t> in base-16 digits and S*/T* the base-16 digit rows of
             # ||s||^2 / ||t||^2.


def _lift_rows(p, is_src):
    """26 paired factor rows: A half then B half (11 each)."""
    c = p.astype(np.int64)
    hi = [c[:, d] >> 4 for d in range(3)]      # a (src) / c (tgt)
    lo = [c[:, d] & 15 for d in range(3)]      # b (src) / e (tgt)
    n2 = (c ** 2).sum(1)
    g = [((n2 >> (4 * i)) & 15).astype(np.float64) for i in range(4)]
    one = np.ones(len(p))
    if is_src:
        A = [16.0 * g[3], 128.0 * one, -16.0 * hi[0], -16.0 * hi[2],
             16.0 * one, -4.0 * hi[1], -4.0 * hi[2], -4.0 * lo[2],
             -2.0 * lo[0], -2.0 * lo[2], one]
        B = [16.0 * g[3], 128.0 * one, -16.0 * hi[1], 16.0 * g[2],
             -4.0 * hi[0], -4.0 * lo[0], -4.0 * lo[1], g[1],
             -2.0 * lo[1], g[0], 16.0 * one]
    else:
        A = [128.0 * one, 16.0 * g[3], 32.0 * hi[0], 32.0 * hi[2],
             16.0 * g[2], 8.0 * lo[1], 8.0 * lo[2], 8.0 * hi[2],
             1.0 * lo[0], 1.0 * lo[2], g[0]]
        B = [128.0 * one, 16.0 * g[3], 32.0 * hi[1], 16.0 * one,
             8.0 * lo[0], 8.0 * hi[0], 8.0 * hi[1], 16.0 * one,
             1.0 * lo[1], one, g[1]]
    return (np.stack(A).astype(np.float32), np.stack(B).astype(np.float32))


def _phi8(s):
    return _lift_rows(s, True)


def _psi8(t):
    return _lift_rows(t, False)


# sentinel column: zero cross terms, ||t||^2 digits of 49151 = (15,15,15,11)
# base-16, so d^2_pad = ||s||^2 + 49151 > any real d^2; never the min.
_PSI_PAD_A = np.array([128.0, 176.0, 0.0, 0.0, 240.0, 0.0, 0.0, 0.0,
                       0.0, 0.0, 15.0], np.float32)
_PSI_PAD_B = np.array([128.0, 176.0, 0.0, 16.0, 0.0, 0.0, 0.0, 16.0,
                       0.0, 1.0, 15.0], np.float32)


def _psi8_padded(cand, w):
    """_psi8 of cand padded to w columns with far sentinels: (A, B) [11, w]."""
    A = np.zeros((K_LIFT, w), np.float32)
    B = np.zeros((K_LIFT, w), np.float32)
    n = min(len(cand), w)
    A[:, :n], B[:, :n] = (x[:, :n] for x in _psi8(cand[:n]))
    A[:, n:] = _PSI_PAD_A[:, None]
    B[:, n:] = _PSI_PAD_B[:, None]
    return A, B


# ------------------------------------------------------- layout + program

def _choose_paths(slot_ws, slot_sm_ok):
    """Greedy per-slot engine-path choice balancing modeled DVE vs Act."""
    path = []
    dve = act = 0.0
    for i, w in enumerate(slot_ws):
        c_tr = w * 1.0417 + 125.0 * w / TILE_COLS
        c_sm = w * 0.8333 + 330.0
        if (w >= SM_MIN_W and slot_sm_ok[i]
                and max(dve, act + c_sm) < max(dve + c_tr, act)):
            path.append("sm")
            act += c_sm
        else:
            path.append("tr")
            dve += c_tr
    return path


def _build_program(layout):
    from concourse import bacc, tile
    import concourse.mybir as mybir

    f32 = mybir.dt.float32
    bf16 = mybir.dt.bfloat16
    nslot = layout["nslot"]
    rhs_tot = layout["rhs_tot"]
    nacc = layout["nacc"]
    nsm = max(layout["nsm"], 1)

    ntr = layout["ntr"]

    nc = bacc.Bacc(None, target_bir_lowering=False)
    fp8 = mybir.dt.float8e4
    lhsT_d = nc.dram_tensor("lhsT", [K_LIFT, nslot * 2 * CH], fp8,
                            kind="ExternalInput")
    rhs_d = nc.dram_tensor("rhs", [K_LIFT, 2 * rhs_tot], fp8,
                           kind="ExternalInput")
    bias_d = nc.dram_tensor("bias", [CH, nsm], f32, kind="ExternalInput")
    out_d = nc.dram_tensor("out", [CH, nacc], f32, kind="ExternalOutput")

    # split the rhs DMA at tile boundaries: small first piece so the first
    # matmuls start early, then ~5 even pieces
    bounds = [0]
    accum = 0
    for t in layout["tiles"]:
        accum += sum(s["w"] for s in t["slots"])
        tgt = rhs_tot / 12 if len(bounds) == 1 else rhs_tot / 6
        if accum - bounds[-1] >= tgt and accum < rhs_tot:
            bounds.append(accum)
    bounds.append(rhs_tot)

    with tile.TileContext(nc) as tc:
        with tc.tile_pool(name="w", bufs=1) as wpool, \
             tc.tile_pool(name="psum", bufs=4, space="PSUM") as ppool:
            lhsT = wpool.tile([K_LIFT, nslot * 2 * CH], fp8)
            rhs = wpool.tile([K_LIFT, 2 * rhs_tot], fp8)
            biasT = wpool.tile([CH, nsm], f32)
            # separate per-engine accumulators: a shared tile would WAW-chain
            # every DVE reduce behind every Act softmin and serialize them
            acc_tr = wpool.tile([CH, max(ntr, 1)], f32)
            acc_sm = wpool.tile([CH, max(nacc - ntr, 1)], f32)
            lsplit = min(24, nslot) * 2 * CH
            nc.sync.dma_start(lhsT[:, :lsplit], lhsT_d[:, :lsplit])
            first = True
            for b0, b1 in zip(bounds[:-1], bounds[1:]):
                if b1 > b0:
                    nc.sync.dma_start(rhs[:, 2 * b0:2 * b1],
                                      rhs_d[:, 2 * b0:2 * b1])
                if first:
                    nc.sync.dma_start(biasT[:], bias_d[:])
                    if lsplit < nslot * 2 * CH:
                        nc.sync.dma_start(lhsT[:, lsplit:], lhsT_d[:, lsplit:])
                    first = False
            for t in layout["tiles"]:
                ps = ppool.tile([CH, TILE_COLS], f32, tag="ps")
                # matmul pieces: <=256 output cols (DoubleRow moving limit),
                # never crossing a 512-col psum bank. start=True zeroes the
                # whole 2KB bank, so only the first piece per bank starts.
                pieces = []
                for s in t["slots"]:
                    i, w, poff, roff = (s["slot"], s["w"], s["poff"],
                                        s["rhs_off"])
                    q = 0
                    while q < w:
                        room = 256 - ((poff + q) % 256)
                        pw = min(256, w - q, room)
                        pieces.append((poff + q, pw, i, roff + q))
                        q += pw
                first_in_bank = {}
                last_in_bank = {}
                for n, (po, pw, _, _) in enumerate(pieces):
                    bank = po // 512
                    first_in_bank.setdefault(bank, n)
                    last_in_bank[bank] = n
                for n, (po, pw, i, ro) in enumerate(pieces):
                    bank = po // 512
                    nc.tensor.matmul(
                        ps[:, po:po + pw],
                        lhsT[:, i * 2 * CH:(i + 1) * 2 * CH].rearrange(
                            "k (two m) -> k two m", two=2),
                        rhs[:, 2 * ro:2 * ro + 2 * pw].rearrange(
                            "k (two n) -> k two n", two=2),
                        start=first_in_bank[bank] == n,
                        stop=last_in_bank[bank] == n,
                        perf_mode=mybir.MatmulPerfMode.DoubleRow,
                    )
                if t["kind"] == "tr":
                    for r in t["runs"]:
                        k, w, poff = r["k"], r["w"], r["poff"]
                        src = ps[:, poff:poff + k * w]
                        if k > 1:
                            src = src.rearrange("p (k w) -> p k w", k=k)
                        nc.vector.tensor_reduce(
                            acc_tr[:, r["acc"]:r["acc"] + k], src,
                            mybir.AxisListType.X, mybir.AluOpType.min,
                        )
                else:
                    for s in t["slots"]:
                        w, poff = s["w"], s["poff"]
                        nc.scalar.activation(
                            ps[:, poff:poff + w], ps[:, poff:poff + w],
                            mybir.ActivationFunctionType.Exp,
                            bias=biasT[:, s["bias"]:s["bias"] + 1],
                            scale=-SM_SCALE,
                            accum_out=acc_sm[:, s["acc"]:s["acc"] + 1],
                        )
            htr = ntr * 2 // 3
            if htr > 0:
                nc.sync.dma_start(out_d[:, :htr], acc_tr[:, :htr])
            if ntr > htr:
                nc.sync.dma_start(out_d[:, htr:ntr], acc_tr[:, htr:])
            nsm_cols = nacc - ntr
            hsm = nsm_cols * 2 // 3
            if hsm > 0:
                nc.sync.dma_start(out_d[:, ntr:ntr + hsm], acc_sm[:, :hsm])
            if nsm_cols > hsm:
                nc.sync.dma_start(out_d[:, ntr + hsm:], acc_sm[:, hsm:])
    nc.compile()
    return nc


# ------------------------------------------------------------------- kernel

def kernel(inputs, targets):
    inputs = np.asarray(inputs)
    targets = np.asarray(targets)
    B = inputs.shape[0]
    out = np.zeros(B, np.float32)

    parts = []
    n_dirs = 0
    dir_of_batch = {}
    for b in range(B):
        a = (inputs[b] > 0).any(0)
        t = (targets[b] > 0).any(0)
        pa = _edge_points(a)
        pt = _edge_points(t)
        if len(pa) == 0 or len(pt) == 0:
            out[b] = np.inf
            continue
        ub_ab = _capped_edt_sq(pt, pa)
        ub_ba = _capped_edt_sq(pa, pt)
        d_ab, d_ba = n_dirs, n_dirs + 1
        n_dirs += 2
        dir_of_batch[b] = (d_ab, d_ba)
        parts += _build_chunks(pa, pt, ub_ab, d_ab)
        parts += _build_chunks(pt, pa, ub_ba, d_ba)

    if not parts:
        return out

    # width of each part, padded to the WQ quantum
    def wof(p):
        return max(WQ, ((len(p["cand"]) + WQ - 1) // WQ) * WQ)

    # LPT packing onto cores by total columns
    order = sorted(range(len(parts)), key=lambda i: -wof(parts[i]))
    per_core = [[] for _ in range(N_CORES)]
    load = [0] * N_CORES
    for i in order:
        k = load.index(min(load))
        per_core[k].append(parts[i])
        load[k] += wof(parts[i])
    for k in range(N_CORES):
        per_core[k].sort(key=wof, reverse=True)

    nslot = max(len(c) for c in per_core)
    slot_ws = []
    slot_sm_ok = []
    for r in range(nslot):
        w = WQ
        ok = True
        for k in range(N_CORES):
            if r < len(per_core[k]):
                w = max(w, wof(per_core[k][r]))
                ok = ok and per_core[k][r]["sm_ok"]
        slot_ws.append(w)
        slot_sm_ok.append(ok)

    key = tuple(slot_ws) + tuple(slot_sm_ok)
    if key not in _prog_cache:
        path = _choose_paths(slot_ws, slot_sm_ok)
        layout = _layout_from_paths(slot_ws, path)
        _prog_cache[key] = (_build_program(layout), layout)
    nc, layout = _prog_cache[key]

    import ml_dtypes
    fp8_np = ml_dtypes.float8_e4m3

    # slot index -> metadata from layout
    slot_info = {}
    for t in layout["tiles"]:
        for s in t["slots"]:
            slot_info[s["slot"]] = (t["kind"], s["rhs_off"], s["acc"],
                                    s.get("bias"), s["poff"])

    def slot_pieces(poff, w):
        pieces = []
        q = 0
        while q < w:
            room = 256 - ((poff + q) % 256)
            pw = min(256, w - q, room)
            pieces.append((q, pw))
            q += pw
        return pieces

    in_maps = []
    for k in range(N_CORES):
        lhsT_np = np.zeros((K_LIFT, nslot * 2 * CH), np.float32)
        rhs_np = np.zeros((K_LIFT, 2 * layout["rhs_tot"]), np.float32)
        bias_np = np.zeros((CH, max(layout["nsm"], 1)), np.float32)
        for r in range(nslot):
            p = per_core[k][r] if r < len(per_core[k]) else per_core[k][0]
            kind, roff, _, bidx, poff = slot_info[r]
            w = layout["slot_ws"][r]
            pA, pB = _phi8(p["src"])
            lhsT_np[:, r * 2 * CH:r * 2 * CH + CH] = pA
            lhsT_np[:, r * 2 * CH + CH:(r + 1) * 2 * CH] = pB
            cA, cB = _psi8_padded(p["cand"], w)
            for q, pw in slot_pieces(poff, w):
                off = 2 * (roff + q)
                rhs_np[:, off:off + pw] = cA[:, q:q + pw]
                rhs_np[:, off + pw:off + 2 * pw] = cB[:, q:q + pw]
            if kind == "sm":
                ub = np.where(np.isfinite(p["ub2"]), p["ub2"], 0.0)
                bias_np[:, bidx] = (SM_SCALE * ub).astype(np.float32)
        in_maps.append({"lhsT": lhsT_np.astype(fp8_np),
                        "rhs": rhs_np.astype(fp8_np),
                        "bias": bias_np})

    from concourse.bass_utils import run_bass_kernel_spmd
    trace = bool(os.environ.get("HD_TRACE"))
    try:
        res = run_bass_kernel_spmd(nc, in_maps, list(range(N_CORES)),
                                   trace=trace)
    except Exception:
        if not trace:
            raise
        res = run_bass_kernel_spmd(nc, in_maps, list(range(N_CORES)),
                                   trace=False)
    if trace and res.exec_time_ns is not None:
        print(f"HW exec time: {res.exec_time_ns} ns")

    # combine parts of each chunk across all cores, then per-source min, max
    groups = {}
    for k in range(N_CORES):
        o = np.asarray(res.results[k]["out"]).astype(np.float64)  # [CH, nacc]
        for r in range(min(nslot, len(per_core[k]))):
            p = per_core[k][r]
            kind, _, aidx, _, _ = slot_info[r]
            col = aidx if kind == "tr" else layout["ntr"] + aidx
            gk = (p["dir"], p["chunk"])
            groups.setdefault(gk, []).append((kind, np.array(o[:, col]), p))
    h2 = np.zeros(n_dirs, np.float64)
    for (d, _), lst in groups.items():
        nreal = lst[0][2]["nreal"]
        mins = np.full(CH, np.inf)
        sm_acc = np.zeros(CH)
        sm_ub = None
        for kind, col, p in lst:
            if kind == "tr":
                mins = np.minimum(mins, col)
            else:
                sm_acc += col
                sm_ub = np.where(np.isfinite(p["ub2"]), p["ub2"], 0.0)
        if sm_ub is not None:
            est = sm_ub - np.log(np.maximum(sm_acc, 1e-30)) / SM_SCALE
            mins = np.minimum(mins, est)
        h2[d] = max(h2[d], float(mins[:nreal].max()))

    for b, (d_ab, d_ba) in dir_of_batch.items():
        out[b] = np.sqrt(np.float32(max(h2[d_ab], h2[d_ba])))
    return out


def _layout_from_paths(slot_ws, path):
    """Shared SPMD layout: tile packing, run grouping, rhs offsets, acc/bias
    column indices, and emission order (tr/sm tiles interleaved)."""
    nslot = len(slot_ws)
    tiles = []
    for kind in ("tr", "sm"):
        cur, cw = [], 0
        for i in range(nslot):
            if path[i] != kind:
                continue
            w = slot_ws[i]
            if cw + w > TILE_COLS and cur:
                tiles.append((kind, cur))
                cur, cw = [], 0
            cur.append(i)
            cw += w
        if cur:
            tiles.append((kind, cur))
    tr_tiles = [t for t in tiles if t[0] == "tr"]
    sm_tiles = [t for t in tiles if t[0] == "sm"]

    # interleave tr/sm tiles weighted by modeled absorb cost so the two
    # engines' work streams stay in lockstep through the whole run
    def tile_cost(t):
        kind, slots = t
        if kind == "tr":
            return sum(slot_ws[i] * 1.0417 for i in slots) + 125.0
        return sum(slot_ws[i] * 0.8333 + 330.0 for i in slots)

    tot_tr = sum(tile_cost(t) for t in tr_tiles) or 1.0
    tot_sm = sum(tile_cost(t) for t in sm_tiles) or 1.0
    order = []
    ntr, nsm = len(tr_tiles), len(sm_tiles)
    ti = si = 0
    cum_tr = cum_sm = 0.0
    for k in range(ntr + nsm):
        take_sm = si < nsm and (ti >= ntr or cum_sm * tot_tr <= cum_tr * tot_sm)
        if take_sm:
            cum_sm += tile_cost(sm_tiles[si])
            order.append(sm_tiles[si]); si += 1
        else:
            cum_tr += tile_cost(tr_tiles[ti])
            order.append(tr_tiles[ti]); ti += 1
    layout = {"slot_ws": slot_ws, "path": path, "tiles": [], "nslot": nslot}
    rhs_off = tr_idx = sm_idx = 0
    for kind, slots in order:
        tile = {"kind": kind, "slots": [], "runs": []}
        poff = 0
        for i in slots:
            w = slot_ws[i]
            tile["slots"].append({"slot": i, "w": w, "poff": poff,
                                  "rhs_off": rhs_off})
            poff += w
            rhs_off += w
        if kind == "tr":
            j = 0
            ss = tile["slots"]
            while j < len(ss):
                k2 = j
                while k2 < len(ss) and ss[k2]["w"] == ss[j]["w"]:
                    k2 += 1
                tile["runs"].append({"poff": ss[j]["poff"], "w": ss[j]["w"],
                                     "k": k2 - j, "acc": tr_idx,
                                     "slots": [s["slot"] for s in ss[j:k2]]})
                for s in ss[j:k2]:
                    s["acc"] = tr_idx
                    tr_idx += 1
                j = k2
        else:
            for s in tile["slots"]:
                s["acc"] = sm_idx
                s["bias"] = sm_idx
                sm_idx += 1
        layout["tiles"].append(tile)
    layout["rhs_tot"] = rhs_off
    layout["ntr"] = tr_idx
    layout["nacc"] = tr_idx + sm_idx
    layout["nsm"] = sm_idx
    return layout
